# revision 15
# baseline (speedup 1.0000x reference)
"""MultiHeadAttention Trainium2 kernel (8 NeuronCores, Bass/Tile).

Problem: B=2, S=2048, D=1024, H=16, DK=64 fp32 MHA (torch-Linear style
projections, softmax attention, output projection).

Sharding: core c = (batch b = c//4, head-group g = c%4); each core handles
4 heads of one batch, entirely in a transposed layout (features on
partitions, sequence on the free axis):
  qhT/khT  = (W_g x^T + b)       [2 pairs x 128, 2048]
  vh       = x_v Wv_g^T          [2048, 4x65] (ones col -> row sums)
  scoresT  = khT^T qhT           per (pair, ktile, qtile) -> PSUM
  expT     = exp(scoresT/8)      ACT -> bf16
  rawT     = vh_aug^T expT       PV matmul; row 64 = softmax denominator
  outT     = rawT[0:64] * (1/rawT[64])
  partialT = woT^T outT          [1024, 2048] fp16 -> DRAM
Host: out[b] = sum_g partialT(b,g)^T + (Wo bv + bo).

v2 pipeline notes (v1 measured 257us, PE 75% busy):
- input DMA rings are serviced round-robin, so unordered loads all land
  at ~21us; ring chaining (chain_iter_dep) staggers wk->wq->xk->xq->xv
  so k-proj starts at ~7us.
- warmup matmuls ramp the PE out of its low p-state during the DMA wait.
- e2 exp tiles cycle through the same 32KB pool slots as the (dead by
  then) xk/xq/xv input tiles, giving 2 units of exp/PV pipelining
  without exceeding SBUF.
- partial output written fp16 (halves tail DMA); host sums in fp32.
"""

import numpy as np

B, S, D, H = 2, 2048, 1024, 16
DK = D // H          # 64
N_CORES = 8
HG = H // 4          # 4 head-groups
HL = 4               # heads per core
FEAT = HL * DK       # 256 per-core features
NQT = S // 512       # 4 query tiles
NKT = S // 128       # 16 key tiles
NDT = D // 128       # 8 contraction tiles (d-model)

DT_QK = "fp16"   # x_q/x_k, Wq/Wk, qhT/khT (score operands)
DT_V = "fp16"    # x_v, Wv
DT_PV = "bf16"   # vh_aug, expT
DT_O = "fp16"    # Wo, outT
N_WARMUP = 24    # PE p-state warmup matmuls during initial DMA wait

_cache = {}


def _np_dt(name):
    if name == "fp16":
        return np.float16
    import ml_dtypes
    return ml_dtypes.bfloat16


def _build():
    import concourse.mybir as mybir
    import concourse.tile as tile
    from concourse import bacc

    fp32 = mybir.dt.float32
    dt_qk = getattr(mybir.dt, "float16" if DT_QK == "fp16" else "bfloat16")
    dt_v = getattr(mybir.dt, "float16" if DT_V == "fp16" else "bfloat16")
    dt_pv = getattr(mybir.dt, "float16" if DT_PV == "fp16" else "bfloat16")
    dt_o = getattr(mybir.dt, "float16" if DT_O == "fp16" else "bfloat16")
    dt_out = mybir.dt.float16

    nc = bacc.Bacc("TRN2", target_bir_lowering=False, debug=False,
                   num_devices=N_CORES)

    xqT = nc.dram_tensor("xqT", [D, S], dt_qk, kind="ExternalInput").ap()
    xkT = nc.dram_tensor("xkT", [D, S], dt_qk, kind="ExternalInput").ap()
    xvT = nc.dram_tensor("xvT", [D, S], dt_v, kind="ExternalInput").ap()
    # weights/biases are pre-swizzled on host so each SBUF partition row
    # is one contiguous 4KB DRAM read: big descriptors keep the DMA queues'
    # round-robin from starving the bulk x transfers
    wqT = nc.dram_tensor("wqT", [128, NDT * FEAT], dt_qk,
                         kind="ExternalInput").ap()
    wkT = nc.dram_tensor("wkT", [128, NDT * FEAT], dt_qk,
                         kind="ExternalInput").ap()
    wvT = nc.dram_tensor("wvT", [128, NDT * FEAT], dt_v,
                         kind="ExternalInput").ap()
    woT = nc.dram_tensor("woT", [128, 2 * D], dt_o, kind="ExternalInput").ap()
    bq2 = nc.dram_tensor("bq2", [128, 2], fp32, kind="ExternalInput").ap()
    bk2 = nc.dram_tensor("bk2", [128, 2], fp32, kind="ExternalInput").ap()
    out_d = nc.dram_tensor("partialT", [D, S], dt_out,
                           kind="ExternalOutput").ap()

    xq_r = xqT.rearrange("(t p) s -> p t s", p=128)
    xk_r = xkT.rearrange("(t p) s -> p t s", p=128)
    xv_r = xvT.rearrange("(t p) s -> p t s", p=128)

    with tile.TileContext(nc) as tc:
        def chain(inst, key):
            # stagger DMA ring groups: rings within a group run in parallel
            # (full HBM bw); later groups start only after the prior group's
            # lane finishes, so early tensors land first.
            try:
                tc.chain_iter_dep(key, inst)
            except Exception:
                pass

        with (
            tc.tile_pool(name="win", bufs=1) as win,
            tc.tile_pool(name="big", bufs=4) as big,
            tc.tile_pool(name="proj", bufs=1) as proj,
            tc.tile_pool(name="pout", bufs=4) as pout,
            tc.tile_pool(name="pnrm", bufs=2) as pnrm,
            tc.tile_pool(name="pp", bufs=2, space="PSUM") as pp,
            tc.tile_pool(name="ps2", bufs=2, space="PSUM") as ps2,
            tc.tile_pool(name="pspv", bufs=2, space="PSUM") as pspv,
        ):
            wdum0 = win.tile([128, 512], dt_qk, tag="wdum")
            junk = win.tile([128, 512], fp32, tag="junk")
            nc.vector.memset(wdum0[:], 0.0)
            # ---- DMA: a ring occupies ~one of the 16 queues, so each x
            # tensor is split into 16 rings (t x partition-half) to saturate
            # HBM; rings are lane-chained xk->xq->xv so xk lands first.
            # Triggers alternate between the SP and ACT HWDGE queues to halve
            # trigger-issue serialization. ----
            wq3 = win.tile([128, NDT, FEAT], dt_qk, tag="wq")
            wk3 = win.tile([128, NDT, FEAT], dt_qk, tag="wk")
            wv3 = win.tile([128, NDT, FEAT], dt_v, tag="wv")
            wo3 = win.tile([128, 2, D], dt_o, tag="wo")
            bq3 = win.tile([128, 2], fp32, tag="bq")
            bk3 = win.tile([128, 2], fp32, tag="bk")

            xk3 = big.tile([128, NDT, S], dt_qk, tag="big")
            xq3 = big.tile([128, NDT, S], dt_qk, tag="big")
            xv3 = big.tile([128, NDT, S], dt_v, tag="big")

            def x_rings(x3, xr):
                for t in range(NDT):
                    for hi, p0 in enumerate((0, 64)):
                        eng = nc.sync if (2 * t + hi) % 2 == 0 else nc.scalar
                        i = eng.dma_start(x3[p0:p0 + 64, t, :],
                                          xr[p0:p0 + 64, t, :])
                        chain(i, f"l{2 * t + hi}")

            x_rings(xk3, xk_r)
            nc.sync.dma_start(wk3[:], wkT.rearrange("p (t f) -> p t f", t=NDT))
            nc.sync.dma_start(wq3[:], wqT.rearrange("p (t f) -> p t f", t=NDT))
            chain(nc.sync.dma_start(bk3[:], bk2), "l0")
            chain(nc.sync.dma_start(bq3[:], bq2), "l1")
            # ACT's exp table preload sits between its xk and xq triggers
            nc.scalar.activation(junk[0:1, 0:1], wdum0[0:1, 0:1],
                                 mybir.ActivationFunctionType.Exp, scale=1.0)
            x_rings(xq3, xq_r)
            x_rings(xv3, xv_r)
            chain(nc.sync.dma_start(
                wv3[:], wvT.rearrange("p (t f) -> p t f", t=NDT)), "l2")
            chain(nc.sync.dma_start(
                wo3[:], woT.rearrange("p (t j) -> p t j", t=2)), "l3")

            # ---- persistent intermediates ----
            qh3 = proj.tile([128, 2, S], dt_qk, tag="qh")   # pair-packed
            kh3 = proj.tile([128, 2, S], dt_qk, tag="kh")
            vha = proj.tile([128, NKT, HL, DK + 1], dt_pv, tag="vha")
            ot3 = proj.tile([128, 2, S], dt_o, tag="outT")
            nc.gpsimd.memset(vha[:, :, :, DK], 1.0)  # ones col -> denominators

            # ---- PE p-state warmup while the first DMAs land ----
            wdum = wdum0
            wu = pp.tile([128, 512], fp32, tag="acc")
            for i in range(N_WARMUP):
                nc.tensor.matmul(wu[:], wdum[:, 0:128], wdum[:],
                                 start=(i == 0), stop=(i == N_WARMUP - 1))
            nc.vector.tensor_copy(junk[:], wu[:])

            # ---- projections: 2 psum accumulators per pass, kt-interleaved
            # so matmuls chase the chunked x DMAs ----
            def qk_pass(x3, w3, b3, dst, m, nn):
                accs = [pp.tile([128, 512], fp32, tag="acc", name=f"acc{n}")
                        for n in nn]
                for kt in range(NDT):
                    for a, n in zip(accs, nn):
                        nc.tensor.matmul(
                            a[:], w3[:, kt, m * 128:(m + 1) * 128],
                            x3[:, kt, n * 512:(n + 1) * 512],
                            start=(kt == 0), stop=(kt == NDT - 1))
                for a, n in zip(accs, nn):
                    nc.vector.tensor_scalar_add(
                        dst[:, m, n * 512:(n + 1) * 512], a[:],
                        b3[:, m:m + 1])

            def v_proj():
                for st in range(NKT):
                    ps = pp.tile([128, 512], fp32, tag="acc")
                    for kt in range(NDT):
                        nc.tensor.matmul(
                            ps[:, 0:256], xv3[:, kt, st * 128:(st + 1) * 128],
                            wv3[:, kt, :],
                            start=(kt == 0), stop=(kt == NDT - 1))
                    nc.vector.tensor_copy(vha[:, st, :, 0:DK],
                                          ps[:, 0:256])

            def attn_scores(qt, hp, e2u):
                for kt in range(NKT):
                    s2 = ps2.tile([128, 1024], fp32, tag="s2")
                    nc.tensor.matmul(
                        s2[:, 0:512],
                        kh3[0:64, hp, kt * 128:(kt + 1) * 128],
                        qh3[0:64, hp, qt * 512:(qt + 1) * 512],
                        start=True, stop=True)
                    nc.tensor.matmul(
                        s2[:, 512:1024],
                        kh3[64:128, hp, kt * 128:(kt + 1) * 128],
                        qh3[64:128, hp, qt * 512:(qt + 1) * 512],
                        start=True, stop=True)
                    nc.scalar.activation(
                        e2u[:, kt, :], s2[:],
                        mybir.ActivationFunctionType.Exp, scale=0.125)

            def attn_pv(qt, hp, e2u):
                pva = pspv.tile([DK + 1, 512], fp32, tag="pv")
                pvb = pspv.tile([DK + 1, 512], fp32, tag="pv")
                for kt in range(NKT):
                    nc.tensor.matmul(
                        pva[:], vha[:, kt, 2 * hp, :], e2u[:, kt, 0:512],
                        start=(kt == 0), stop=(kt == NKT - 1))
                    nc.tensor.matmul(
                        pvb[:], vha[:, kt, 2 * hp + 1, :],
                        e2u[:, kt, 512:1024],
                        start=(kt == 0), stop=(kt == NKT - 1))
                for pv, half in ((pva, 0), (pvb, 1)):
                    # copy the whole accumulator to SBUF first: frees the
                    # PSUM bank for the next unit's PV in ~0.7us instead of
                    # after the full normalize chain; DVE cost is the same
                    # (free-size bound) and custom DVE ops need SBUF anyway.
                    pvs = pnrm.tile([DK + 1, 512], fp32, tag="pvs")
                    nc.vector.tensor_copy(pvs[:], pv[:])
                    # custom-DVE recip needs a base-partition-0 input; BIR
                    # also requires partition starts in {0,32,64,96}, so the
                    # denominator row (partition 64) is staged via srow
                    srow = pnrm.tile([1, 512], fp32, tag="srow")
                    nc.vector.tensor_copy(srow[:], pvs[DK:DK + 1, :])
                    inv = pnrm.tile([1, 512], fp32, tag="inv")
                    nc.vector.reciprocal_approx_fast(inv[:], srow[:])
                    invb = pnrm.tile([64, 512], fp32, tag="invb")
                    nc.gpsimd.partition_broadcast(invb[:], inv[:])
                    nc.vector.tensor_tensor(
                        ot3[half * 64:(half + 1) * 64, hp,
                            qt * 512:(qt + 1) * 512],
                        pvs[0:DK, :], invb[:], mybir.AluOpType.mult)

            def oproj(qt):
                for jt in range(NDT):
                    ps = pp.tile([128, 512], fp32, tag="acc")
                    for m in range(2):
                        nc.tensor.matmul(
                            ps[:], wo3[:, m, jt * 128:(jt + 1) * 128],
                            ot3[:, m, qt * 512:(qt + 1) * 512],
                            start=(m == 0), stop=(m == 1))
                    po = pout.tile([128, 512], dt_out, tag="po")
                    nc.vector.tensor_copy(po[:], ps[:])
                    nc.sync.dma_start(
                        out_d[jt * 128:(jt + 1) * 128,
                              qt * 512:(qt + 1) * 512], po[:])

            def e2tile(name):
                return big.tile([128, NKT, 1024], dt_pv, tag="big", name=name)

            # ---- emission order == per-engine execution order ----
            for m in range(2):                       # k-proj (all pairs)
                for nn in ((0, 1), (2, 3)):
                    qk_pass(xk3, wk3, bk3, kh3, m, nn)
            qk_pass(xq3, wq3, bq3, qh3, 0, (0,))     # q-proj heads only
            qk_pass(xq3, wq3, bq3, qh3, 1, (0,))
            e00 = e2tile("e00")
            attn_scores(0, 0, e00)                   # ACT starts here
            e01 = e2tile("e01")
            attn_scores(0, 1, e01)
            for m in range(2):                       # q-proj remainder
                qk_pass(xq3, wq3, bq3, qh3, m, (1, 2))
                qk_pass(xq3, wq3, bq3, qh3, m, (3,))
            v_proj()
            attn_pv(0, 0, e00)
            e10 = e2tile("e10")
            attn_scores(1, 0, e10)
            attn_pv(0, 1, e01)
            oproj(0)
            prev = {(1, 0): e10}
            for qt in range(1, NQT):
                e_b = e2tile(f"e{qt}1")
                attn_scores(qt, 1, e_b)
                attn_pv(qt, 0, prev[(qt, 0)])
                if qt < NQT - 1:
                    e_a = e2tile(f"e{qt + 1}0")
                    attn_scores(qt + 1, 0, e_a)
                    prev[(qt + 1, 0)] = e_a
                attn_pv(qt, 1, e_b)
                if qt == NQT - 1:
                    wu2 = pp.tile([128, 512], fp32, tag="acc", name="wu2")
                    for i in range(10):
                        nc.tensor.matmul(wu2[:], wdum[:, 0:128], wdum[:],
                                         start=(i == 0), stop=(i == 9))
                    nc.vector.tensor_copy(junk[:], wu2[:])
                oproj(qt)

    nc.compile()
    return nc


def kernel(q, k, v, Wq, bq, Wk, bk, Wv, bv, Wo, bo, _trace=False):
    from concourse import bass_utils

    if "nc" not in _cache:
        _cache["nc"] = _build()
    nc = _cache["nc"]

    q = np.asarray(q, np.float32)
    k = np.asarray(k, np.float32)
    v = np.asarray(v, np.float32)
    Wq = np.asarray(Wq, np.float32)
    Wk = np.asarray(Wk, np.float32)
    Wv = np.asarray(Wv, np.float32)
    Wo = np.asarray(Wo, np.float32)
    bq = np.asarray(bq, np.float32)
    bk = np.asarray(bk, np.float32)
    bv = np.asarray(bv, np.float32)
    bo = np.asarray(bo, np.float32)

    d_qk, d_v, d_o = _np_dt(DT_QK), _np_dt(DT_V), _np_dt(DT_O)
    xT = {}
    for b in range(B):
        xT[("q", b)] = np.ascontiguousarray(q[b].T).astype(d_qk)
        xT[("k", b)] = np.ascontiguousarray(k[b].T).astype(d_qk)
        xT[("v", b)] = np.ascontiguousarray(v[b].T).astype(d_v)
    wT = {}
    for g in range(HG):
        sl = slice(g * FEAT, (g + 1) * FEAT)
        # device expects [128, t*f]: row p holds, for each contraction
        # tile t, the weight row t*128+p — one contiguous 4KB DMA descriptor
        def swz(a, t):     # [t*128, f] -> [128, t*f]
            f = a.shape[1]
            return np.ascontiguousarray(
                a.reshape(t, 128, f).transpose(1, 0, 2).reshape(128, t * f))
        wT[("q", g)] = swz(np.ascontiguousarray(Wq[sl, :].T), NDT).astype(d_qk)
        wT[("k", g)] = swz(np.ascontiguousarray(Wk[sl, :].T), NDT).astype(d_qk)
        wT[("v", g)] = swz(np.ascontiguousarray(Wv[sl, :].T), NDT).astype(d_v)
        wT[("o", g)] = swz(np.ascontiguousarray(Wo[:, sl].T), 2).astype(d_o)

    in_maps = []
    for c in range(N_CORES):
        b, g = divmod(c, HG)
        sl = slice(g * FEAT, (g + 1) * FEAT)
        in_maps.append({
            "xqT": xT[("q", b)], "xkT": xT[("k", b)], "xvT": xT[("v", b)],
            "wqT": wT[("q", g)], "wkT": wT[("k", g)], "wvT": wT[("v", g)],
            "woT": wT[("o", g)],
            "bq2": np.ascontiguousarray(bq[sl].reshape(2, 128).T),
            "bk2": np.ascontiguousarray(bk[sl].reshape(2, 128).T),
        })

    kwargs = {}
    if _trace:
        _install_profile_shim()
        kwargs = dict(trace=True, trace_cores=list(range(N_CORES)))
    res = bass_utils.run_bass_kernel_spmd(
        nc, in_maps, core_ids=list(range(N_CORES)), **kwargs)
    _cache["last_results"] = res

    final_bias = (Wo @ bv + bo).astype(np.float32)  # attn rows sum to 1
    out = np.empty((B, S, D), np.float32)
    for b in range(B):
        acc = res.results[b * HG]["partialT"].astype(np.float32)
        for g in range(1, HG):
            acc += res.results[b * HG + g]["partialT"].astype(np.float32)
        out[b] = acc.T + final_bias
    return out


def _install_profile_shim():
    """Provide antenv.axon_hooks so trace=True works under axon."""
    import sys
    import types

    import antenv

    if "antenv.axon_hooks" in sys.modules:
        return
    mod = types.ModuleType("antenv.axon_hooks")
    mod._hook = None
    mod.set_axon_ntff_profile_hook = lambda h: setattr(mod, "_hook", h)
    mod.get_axon_ntff_profile_hook = lambda: mod._hook
    sys.modules["antenv.axon_hooks"] = mod
    antenv.axon_hooks = mod
    try:
        from trn_agent_boot.trn_boot import _ntff_profile_via_ctypes
        mod.set_axon_ntff_profile_hook(
            _ntff_profile_via_ctypes("/opt/axon/libaxon_pjrt.so"))
    except Exception:
        pass


# revision 17
# speedup vs baseline: 1.1852x; 1.1852x over previous
"""MultiHeadAttention Trainium2 kernel (8 NeuronCores, Bass/Tile).

Problem: B=2, S=2048, D=1024, H=16, DK=64 fp32 MHA (torch-Linear style
projections, softmax attention, output projection).

Sharding: core c = (batch b = c//4, head-group g = c%4); each core handles
4 heads of one batch, entirely in a transposed layout (features on
partitions, sequence on the free axis):
  qhT/khT  = (W_g x^T + b)       [2 pairs x 128, 2048]
  vh       = x_v Wv_g^T          [2048, 4x65] (ones col -> row sums)
  scoresT  = khT^T qhT           per (pair, ktile, qtile) -> PSUM
  expT     = exp(scoresT/8)      ACT -> bf16
  rawT     = vh_aug^T expT       PV matmul; row 64 = softmax denominator
  outT     = rawT[0:64] * (1/rawT[64])
  partialT = woT^T outT          [1024, 2048] fp16 -> DRAM
Host: out[b] = sum_g partialT(b,g)^T + (Wo bv + bo).

v2 pipeline notes (v1 measured 257us, PE 75% busy):
- input DMA rings are serviced round-robin, so unordered loads all land
  at ~21us; ring chaining (chain_iter_dep) staggers wk->wq->xk->xq->xv
  so k-proj starts at ~7us.
- warmup matmuls ramp the PE out of its low p-state during the DMA wait.
- e2 exp tiles cycle through the same 32KB pool slots as the (dead by
  then) xk/xq/xv input tiles, giving 2 units of exp/PV pipelining
  without exceeding SBUF.
- partial output written fp16 (halves tail DMA); host sums in fp32.
"""

import numpy as np

B, S, D, H = 2, 2048, 1024, 16
DK = D // H          # 64
N_CORES = 8
HG = H // 4          # 4 head-groups
HL = 4               # heads per core
FEAT = HL * DK       # 256 per-core features
NQT = S // 512       # 4 query tiles
NKT = S // 128       # 16 key tiles
NDT = D // 128       # 8 contraction tiles (d-model)

DT_QK = "fp16"   # x_q/x_k, Wq/Wk, qhT/khT (score operands)
DT_V = "fp16"    # x_v, Wv
DT_PV = "bf16"   # vh_aug, expT
DT_O = "fp16"    # Wo, outT
N_WARMUP = 24    # PE p-state warmup matmuls during initial DMA wait

_cache = {}


def _np_dt(name):
    if name == "fp16":
        return np.float16
    import ml_dtypes
    return ml_dtypes.bfloat16


def _build():
    import concourse.mybir as mybir
    import concourse.tile as tile
    from concourse import bacc

    fp32 = mybir.dt.float32
    dt_qk = getattr(mybir.dt, "float16" if DT_QK == "fp16" else "bfloat16")
    dt_v = getattr(mybir.dt, "float16" if DT_V == "fp16" else "bfloat16")
    dt_pv = getattr(mybir.dt, "float16" if DT_PV == "fp16" else "bfloat16")
    dt_o = getattr(mybir.dt, "float16" if DT_O == "fp16" else "bfloat16")
    dt_out = mybir.dt.float16

    nc = bacc.Bacc("TRN2", target_bir_lowering=False, debug=False,
                   num_devices=N_CORES)

    # all inputs host-swizzled to [128, ...] so each SBUF partition row is
    # ONE contiguous DRAM read. DMA queues round-robin per DESCRIPTOR, so
    # descriptor size acts as priority: x uses 32KB descriptors, weights
    # ~8KB. qk biases ride in the qk weight buffer (fp32 bit-packed into 4
    # trailing fp16 columns, bitcast on device) to avoid tiny descriptors.
    xqT = nc.dram_tensor("xqT", [128, NDT * S], dt_qk,
                         kind="ExternalInput").ap()
    xkT = nc.dram_tensor("xkT", [128, NDT * S], dt_qk,
                         kind="ExternalInput").ap()
    xvT = nc.dram_tensor("xvT", [128, NDT * S], dt_v,
                         kind="ExternalInput").ap()
    wqkT = nc.dram_tensor("wqkT", [128, 2 * NDT * FEAT + 8], dt_qk,
                          kind="ExternalInput").ap()
    wvoT = nc.dram_tensor("wvoT", [128, NDT * FEAT + 2 * D], dt_v,
                          kind="ExternalInput").ap()
    out_d = nc.dram_tensor("partialT", [D, S], dt_out,
                           kind="ExternalOutput").ap()

    xq_r = xqT.rearrange("p (t s) -> p t s", t=NDT)
    xk_r = xkT.rearrange("p (t s) -> p t s", t=NDT)
    xv_r = xvT.rearrange("p (t s) -> p t s", t=NDT)

    with tile.TileContext(nc) as tc:
        def chain(inst, key):
            # stagger DMA ring groups: rings within a group run in parallel
            # (full HBM bw); later groups start only after the prior group's
            # lane finishes, so early tensors land first.
            try:
                tc.chain_iter_dep(key, inst)
            except Exception:
                pass

        with (
            tc.tile_pool(name="win", bufs=1) as win,
            tc.tile_pool(name="big", bufs=4) as big,
            tc.tile_pool(name="proj", bufs=1) as proj,
            tc.tile_pool(name="pout", bufs=4) as pout,
            tc.tile_pool(name="pnrm", bufs=2) as pnrm,
            tc.tile_pool(name="pp", bufs=2, space="PSUM") as pp,
            tc.tile_pool(name="ps2", bufs=2, space="PSUM") as ps2,
            tc.tile_pool(name="pspv", bufs=2, space="PSUM") as pspv,
        ):
            wdum0 = win.tile([128, 512], dt_qk, tag="wdum")
            junk = win.tile([128, 512], fp32, tag="junk")
            nc.vector.memset(wdum0[:], 0.0)
            # ---- DMA: one big-descriptor ring per tensor; 3-hop chain
            # xk -> xq -> xv -> wvo so earlier-needed tensors get the full
            # link. wqk (weights+biases) rides unchained beside xk. ----
            wqk = win.tile([128, 2 * NDT * FEAT + 8], dt_qk, tag="wqk")
            wvo = win.tile([128, NDT * FEAT + 2 * D], dt_v, tag="wvo")

            xk3 = big.tile([128, NDT, S], dt_qk, tag="big")
            xq3 = big.tile([128, NDT, S], dt_qk, tag="big")
            xv3 = big.tile([128, NDT, S], dt_v, tag="big")

            chain(nc.sync.dma_start(xk3[:], xk_r), "l0")
            nc.sync.dma_start(wqk[:], wqkT)
            nc.scalar.activation(junk[0:1, 0:1], wdum0[0:1, 0:1],
                                 mybir.ActivationFunctionType.Exp, scale=1.0)
            chain(nc.sync.dma_start(xq3[:], xq_r), "l0")
            chain(nc.sync.dma_start(xv3[:], xv_r), "l0")
            chain(nc.sync.dma_start(wvo[:], wvoT), "l0")

            wk3 = wqk[:, 0:NDT * FEAT].rearrange("p (t f) -> p t f", t=NDT)
            wq3 = wqk[:, NDT * FEAT:2 * NDT * FEAT].rearrange(
                "p (t f) -> p t f", t=NDT)
            bk3 = wqk[:, 2 * NDT * FEAT:2 * NDT * FEAT + 4].bitcast(fp32)
            bq3 = wqk[:, 2 * NDT * FEAT + 4:2 * NDT * FEAT + 8].bitcast(fp32)
            wv3 = wvo[:, 0:NDT * FEAT].rearrange("p (t f) -> p t f", t=NDT)
            wo3 = wvo[:, NDT * FEAT:].rearrange("p (t j) -> p t j", t=2)

            # ---- persistent intermediates ----
            qh3 = proj.tile([128, 2, S], dt_qk, tag="qh")   # pair-packed
            kh3 = proj.tile([128, 2, S], dt_qk, tag="kh")
            vha = proj.tile([128, NKT, HL, DK + 1], dt_pv, tag="vha")
            ot3 = proj.tile([128, 2, S], dt_o, tag="outT")
            nc.gpsimd.memset(vha[:, :, :, DK], 1.0)  # ones col -> denominators

            # ---- PE p-state warmup while the first DMAs land ----
            wdum = wdum0
            wu = pp.tile([128, 512], fp32, tag="acc")
            for i in range(N_WARMUP):
                nc.tensor.matmul(wu[:], wdum[:, 0:128], wdum[:],
                                 start=(i == 0), stop=(i == N_WARMUP - 1))
            nc.vector.tensor_copy(junk[:], wu[:])

            # ---- projections: 2 psum accumulators per pass, kt-interleaved
            # so matmuls chase the chunked x DMAs ----
            def qk_pass(x3, w3, b3, dst, m, nn):
                accs = [pp.tile([128, 512], fp32, tag="acc", name=f"acc{n}")
                        for n in nn]
                for kt in range(NDT):
                    for a, n in zip(accs, nn):
                        nc.tensor.matmul(
                            a[:], w3[:, kt, m * 128:(m + 1) * 128],
                            x3[:, kt, n * 512:(n + 1) * 512],
                            start=(kt == 0), stop=(kt == NDT - 1))
                for a, n in zip(accs, nn):
                    nc.vector.tensor_scalar_add(
                        dst[:, m, n * 512:(n + 1) * 512], a[:],
                        b3[:, m:m + 1])

            def v_proj():
                for st in range(NKT):
                    ps = pp.tile([128, 512], fp32, tag="acc")
                    for kt in range(NDT):
                        nc.tensor.matmul(
                            ps[:, 0:256], xv3[:, kt, st * 128:(st + 1) * 128],
                            wv3[:, kt, :],
                            start=(kt == 0), stop=(kt == NDT - 1))
                    nc.vector.tensor_copy(vha[:, st, :, 0:DK],
                                          ps[:, 0:256])

            def attn_scores(qt, hp, e2u):
                for kt in range(NKT):
                    s2 = ps2.tile([128, 1024], fp32, tag="s2")
                    nc.tensor.matmul(
                        s2[:, 0:512],
                        kh3[0:64, hp, kt * 128:(kt + 1) * 128],
                        qh3[0:64, hp, qt * 512:(qt + 1) * 512],
                        start=True, stop=True)
                    nc.tensor.matmul(
                        s2[:, 512:1024],
                        kh3[64:128, hp, kt * 128:(kt + 1) * 128],
                        qh3[64:128, hp, qt * 512:(qt + 1) * 512],
                        start=True, stop=True)
                    nc.scalar.activation(
                        e2u[:, kt, :], s2[:],
                        mybir.ActivationFunctionType.Exp, scale=0.125)

            def attn_pv(qt, hp, e2u):
                pva = pspv.tile([DK + 1, 512], fp32, tag="pv")
                pvb = pspv.tile([DK + 1, 512], fp32, tag="pv")
                for kt in range(NKT):
                    nc.tensor.matmul(
                        pva[:], vha[:, kt, 2 * hp, :], e2u[:, kt, 0:512],
                        start=(kt == 0), stop=(kt == NKT - 1))
                    nc.tensor.matmul(
                        pvb[:], vha[:, kt, 2 * hp + 1, :],
                        e2u[:, kt, 512:1024],
                        start=(kt == 0), stop=(kt == NKT - 1))
                for pv, half in ((pva, 0), (pvb, 1)):
                    # copy the whole accumulator to SBUF first: frees the
                    # PSUM bank for the next unit's PV in ~0.7us instead of
                    # after the full normalize chain; DVE cost is the same
                    # (free-size bound) and custom DVE ops need SBUF anyway.
                    pvs = pnrm.tile([DK + 1, 512], fp32, tag="pvs")
                    nc.vector.tensor_copy(pvs[:], pv[:])
                    # custom-DVE recip needs a base-partition-0 input; BIR
                    # also requires partition starts in {0,32,64,96}, so the
                    # denominator row (partition 64) is staged via srow
                    srow = pnrm.tile([1, 512], fp32, tag="srow")
                    nc.vector.tensor_copy(srow[:], pvs[DK:DK + 1, :])
                    inv = pnrm.tile([1, 512], fp32, tag="inv")
                    nc.vector.reciprocal_approx_fast(inv[:], srow[:])
                    invb = pnrm.tile([64, 512], fp32, tag="invb")
                    nc.gpsimd.partition_broadcast(invb[:], inv[:])
                    nc.vector.tensor_tensor(
                        ot3[half * 64:(half + 1) * 64, hp,
                            qt * 512:(qt + 1) * 512],
                        pvs[0:DK, :], invb[:], mybir.AluOpType.mult)

            def oproj(qt):
                for jt in range(NDT):
                    ps = pp.tile([128, 512], fp32, tag="acc")
                    for m in range(2):
                        nc.tensor.matmul(
                            ps[:], wo3[:, m, jt * 128:(jt + 1) * 128],
                            ot3[:, m, qt * 512:(qt + 1) * 512],
                            start=(m == 0), stop=(m == 1))
                    po = pout.tile([128, 512], dt_out, tag="po")
                    nc.vector.tensor_copy(po[:], ps[:])
                    nc.sync.dma_start(
                        out_d[jt * 128:(jt + 1) * 128,
                              qt * 512:(qt + 1) * 512], po[:])

            def e2tile(name):
                return big.tile([128, NKT, 1024], dt_pv, tag="big", name=name)

            # ---- emission order == per-engine execution order ----
            for m in range(2):                       # k-proj (all pairs)
                for nn in ((0, 1), (2, 3)):
                    qk_pass(xk3, wk3, bk3, kh3, m, nn)
            qk_pass(xq3, wq3, bq3, qh3, 0, (0,))     # q-proj heads only
            qk_pass(xq3, wq3, bq3, qh3, 1, (0,))
            e00 = e2tile("e00")
            attn_scores(0, 0, e00)                   # ACT starts here
            e01 = e2tile("e01")
            attn_scores(0, 1, e01)
            for m in range(2):                       # q-proj remainder
                qk_pass(xq3, wq3, bq3, qh3, m, (1, 2))
                qk_pass(xq3, wq3, bq3, qh3, m, (3,))
            v_proj()
            attn_pv(0, 0, e00)
            e10 = e2tile("e10")
            attn_scores(1, 0, e10)
            attn_pv(0, 1, e01)
            oproj(0)
            prev = {(1, 0): e10}
            for qt in range(1, NQT):
                e_b = e2tile(f"e{qt}1")
                attn_scores(qt, 1, e_b)
                attn_pv(qt, 0, prev[(qt, 0)])
                if qt < NQT - 1:
                    e_a = e2tile(f"e{qt + 1}0")
                    attn_scores(qt + 1, 0, e_a)
                    prev[(qt + 1, 0)] = e_a
                attn_pv(qt, 1, e_b)
                if qt == NQT - 1:
                    wu2 = pp.tile([128, 512], fp32, tag="acc", name="wu2")
                    for i in range(10):
                        nc.tensor.matmul(wu2[:], wdum[:, 0:128], wdum[:],
                                         start=(i == 0), stop=(i == 9))
                    nc.vector.tensor_copy(junk[:], wu2[:])
                oproj(qt)

    nc.compile()
    return nc


def kernel(q, k, v, Wq, bq, Wk, bk, Wv, bv, Wo, bo, _trace=False):
    from concourse import bass_utils

    if "nc" not in _cache:
        _cache["nc"] = _build()
    nc = _cache["nc"]

    q = np.asarray(q, np.float32)
    k = np.asarray(k, np.float32)
    v = np.asarray(v, np.float32)
    Wq = np.asarray(Wq, np.float32)
    Wk = np.asarray(Wk, np.float32)
    Wv = np.asarray(Wv, np.float32)
    Wo = np.asarray(Wo, np.float32)
    bq = np.asarray(bq, np.float32)
    bk = np.asarray(bk, np.float32)
    bv = np.asarray(bv, np.float32)
    bo = np.asarray(bo, np.float32)

    d_qk, d_v, d_o = _np_dt(DT_QK), _np_dt(DT_V), _np_dt(DT_O)

    def swz(a, t):     # [t*128, f] -> [128, t*f], rows contiguous in DRAM
        f = a.shape[1]
        return np.ascontiguousarray(
            a.reshape(t, 128, f).transpose(1, 0, 2).reshape(128, t * f))

    xT = {}
    for b in range(B):
        xT[("q", b)] = swz(np.ascontiguousarray(q[b].T), NDT).astype(d_qk)
        xT[("k", b)] = swz(np.ascontiguousarray(k[b].T), NDT).astype(d_qk)
        xT[("v", b)] = swz(np.ascontiguousarray(v[b].T), NDT).astype(d_v)
    wT = {}
    for g in range(HG):
        sl = slice(g * FEAT, (g + 1) * FEAT)
        wk_s = swz(np.ascontiguousarray(Wk[sl, :].T), NDT).astype(d_qk)
        wq_s = swz(np.ascontiguousarray(Wq[sl, :].T), NDT).astype(d_qk)
        bk_s = np.ascontiguousarray(
            bk[sl].astype(np.float32).reshape(2, 128).T).view(np.uint16)
        bq_s = np.ascontiguousarray(
            bq[sl].astype(np.float32).reshape(2, 128).T).view(np.uint16)
        wqk = np.concatenate(
            [wk_s.view(np.uint16), wq_s.view(np.uint16), bk_s, bq_s],
            axis=1).view(d_qk)
        wv_s = swz(np.ascontiguousarray(Wv[sl, :].T), NDT).astype(d_v)
        wo_s = swz(np.ascontiguousarray(Wo[:, sl].T), 2).astype(d_o)
        wT[("qk", g)] = np.ascontiguousarray(wqk)
        wT[("vo", g)] = np.ascontiguousarray(
            np.concatenate([wv_s, wo_s], axis=1))

    in_maps = []
    for c in range(N_CORES):
        b, g = divmod(c, HG)
        in_maps.append({
            "xqT": xT[("q", b)], "xkT": xT[("k", b)], "xvT": xT[("v", b)],
            "wqkT": wT[("qk", g)], "wvoT": wT[("vo", g)],
        })

    kwargs = {}
    if _trace:
        _install_profile_shim()
        kwargs = dict(trace=True, trace_cores=list(range(N_CORES)))
    res = bass_utils.run_bass_kernel_spmd(
        nc, in_maps, core_ids=list(range(N_CORES)), **kwargs)
    _cache["last_results"] = res

    final_bias = (Wo @ bv + bo).astype(np.float32)  # attn rows sum to 1
    out = np.empty((B, S, D), np.float32)
    for b in range(B):
        acc = res.results[b * HG]["partialT"].astype(np.float32)
        for g in range(1, HG):
            acc += res.results[b * HG + g]["partialT"].astype(np.float32)
        out[b] = acc.T + final_bias
    return out


def _install_profile_shim():
    """Provide antenv.axon_hooks so trace=True works under axon."""
    import sys
    import types

    import antenv

    if "antenv.axon_hooks" in sys.modules:
        return
    mod = types.ModuleType("antenv.axon_hooks")
    mod._hook = None
    mod.set_axon_ntff_profile_hook = lambda h: setattr(mod, "_hook", h)
    mod.get_axon_ntff_profile_hook = lambda: mod._hook
    sys.modules["antenv.axon_hooks"] = mod
    antenv.axon_hooks = mod
    try:
        from trn_agent_boot.trn_boot import _ntff_profile_via_ctypes
        mod.set_axon_ntff_profile_hook(
            _ntff_profile_via_ctypes("/opt/axon/libaxon_pjrt.so"))
    except Exception:
        pass


# revision 18
# speedup vs baseline: 1.2156x; 1.0257x over previous
"""MultiHeadAttention Trainium2 kernel (8 NeuronCores, Bass/Tile).

Problem: B=2, S=2048, D=1024, H=16, DK=64 fp32 MHA (torch-Linear style
projections, softmax attention, output projection).

Sharding: core c = (batch b = c//4, head-group g = c%4); each core handles
4 heads of one batch, entirely in a transposed layout (features on
partitions, sequence on the free axis):
  qhT/khT  = (W_g x^T + b)       [2 pairs x 128, 2048]
  vh       = x_v Wv_g^T          [2048, 4x65] (ones col -> row sums)
  scoresT  = khT^T qhT           per (pair, ktile, qtile) -> PSUM
  expT     = exp(scoresT/8)      ACT -> bf16
  rawT     = vh_aug^T expT       PV matmul; row 64 = softmax denominator
  outT     = rawT[0:64] * (1/rawT[64])
  partialT = woT^T outT          [1024, 2048] fp16 -> DRAM
Host: out[b] = sum_g partialT(b,g)^T + (Wo bv + bo).

v2 pipeline notes (v1 measured 257us, PE 75% busy):
- input DMA rings are serviced round-robin, so unordered loads all land
  at ~21us; ring chaining (chain_iter_dep) staggers wk->wq->xk->xq->xv
  so k-proj starts at ~7us.
- warmup matmuls ramp the PE out of its low p-state during the DMA wait.
- e2 exp tiles cycle through the same 32KB pool slots as the (dead by
  then) xk/xq/xv input tiles, giving 2 units of exp/PV pipelining
  without exceeding SBUF.
- partial output written fp16 (halves tail DMA); host sums in fp32.
"""

import numpy as np

B, S, D, H = 2, 2048, 1024, 16
DK = D // H          # 64
N_CORES = 8
HG = H // 4          # 4 head-groups
HL = 4               # heads per core
FEAT = HL * DK       # 256 per-core features
NQT = S // 512       # 4 query tiles
NKT = S // 128       # 16 key tiles
NDT = D // 128       # 8 contraction tiles (d-model)

DT_QK = "fp16"   # x_q/x_k, Wq/Wk, qhT/khT (score operands)
DT_V = "fp16"    # x_v, Wv
DT_PV = "bf16"   # vh_aug, expT
DT_O = "fp16"    # Wo, outT
N_WARMUP = 24    # PE p-state warmup matmuls during initial DMA wait

_cache = {}


def _np_dt(name):
    if name == "fp16":
        return np.float16
    import ml_dtypes
    return ml_dtypes.bfloat16


def _build():
    import concourse.mybir as mybir
    import concourse.tile as tile
    from concourse import bacc

    fp32 = mybir.dt.float32
    dt_qk = getattr(mybir.dt, "float16" if DT_QK == "fp16" else "bfloat16")
    dt_v = getattr(mybir.dt, "float16" if DT_V == "fp16" else "bfloat16")
    dt_pv = getattr(mybir.dt, "float16" if DT_PV == "fp16" else "bfloat16")
    dt_o = getattr(mybir.dt, "float16" if DT_O == "fp16" else "bfloat16")
    dt_out = mybir.dt.float16

    nc = bacc.Bacc("TRN2", target_bir_lowering=False, debug=False,
                   num_devices=N_CORES)

    # all inputs host-swizzled to [128, ...] so each SBUF partition row is
    # ONE contiguous DRAM read. DMA queues round-robin per DESCRIPTOR, so
    # descriptor size acts as priority: x uses 32KB descriptors, weights
    # ~8KB. qk biases ride in the qk weight buffer (fp32 bit-packed into 4
    # trailing fp16 columns, bitcast on device) to avoid tiny descriptors.
    xqT = nc.dram_tensor("xqT", [128, NDT * S], dt_qk,
                         kind="ExternalInput").ap()
    xkT = nc.dram_tensor("xkT", [128, NDT * S], dt_qk,
                         kind="ExternalInput").ap()
    xvT = nc.dram_tensor("xvT", [128, NDT * S], dt_v,
                         kind="ExternalInput").ap()
    wqkT = nc.dram_tensor("wqkT", [128, 2 * NDT * FEAT + 8], dt_qk,
                          kind="ExternalInput").ap()
    wvoT = nc.dram_tensor("wvoT", [128, NDT * FEAT + 2 * D], dt_v,
                          kind="ExternalInput").ap()
    # output layout [qt, p, jt, s]: each partition row is one contiguous
    # 8KB write (fewer, bigger DMA descriptors); host re-transposes
    out_d = nc.dram_tensor("partialT", [NQT, 128, NDT, 512], dt_out,
                           kind="ExternalOutput").ap()

    xq_r = xqT.rearrange("p (t s) -> p t s", t=NDT)
    xk_r = xkT.rearrange("p (t s) -> p t s", t=NDT)
    xv_r = xvT.rearrange("p (t s) -> p t s", t=NDT)

    with tile.TileContext(nc) as tc:
        def chain(inst, key):
            # stagger DMA ring groups: rings within a group run in parallel
            # (full HBM bw); later groups start only after the prior group's
            # lane finishes, so early tensors land first.
            try:
                tc.chain_iter_dep(key, inst)
            except Exception:
                pass

        with (
            tc.tile_pool(name="win", bufs=1) as win,
            tc.tile_pool(name="big", bufs=4) as big,
            tc.tile_pool(name="proj", bufs=1) as proj,
            tc.tile_pool(name="pout", bufs=1) as pout,
            tc.tile_pool(name="pnrm", bufs=2) as pnrm,
            tc.tile_pool(name="pp", bufs=2, space="PSUM") as pp,
            tc.tile_pool(name="ps2", bufs=2, space="PSUM") as ps2,
            tc.tile_pool(name="pspv", bufs=2, space="PSUM") as pspv,
        ):
            wdum0 = win.tile([128, 512], dt_qk, tag="wdum")
            junk = win.tile([128, 512], dt_qk, tag="junk")
            nc.vector.memset(wdum0[:], 0.0)
            # ---- DMA: one big-descriptor ring per tensor; 3-hop chain
            # xk -> xq -> xv -> wvo so earlier-needed tensors get the full
            # link. wqk (weights+biases) rides unchained beside xk. ----
            wqk = win.tile([128, 2 * NDT * FEAT + 8], dt_qk, tag="wqk")
            wvo = win.tile([128, NDT * FEAT + 2 * D], dt_v, tag="wvo")

            xk3 = big.tile([128, NDT, S], dt_qk, tag="big")
            xq3 = big.tile([128, NDT, S], dt_qk, tag="big")
            xv3 = big.tile([128, NDT, S], dt_v, tag="big")

            chain(nc.sync.dma_start(wqk[:], wqkT), "l0")
            nc.scalar.activation(junk[0:1, 0:1], wdum0[0:1, 0:1],
                                 mybir.ActivationFunctionType.Exp, scale=1.0)
            for x3, xr in ((xk3, xk_r), (xq3, xq_r), (xv3, xv_r)):
                for t0 in (0, 4):
                    chain(nc.sync.dma_start(x3[:, t0:t0 + 4, :],
                                            xr[:, t0:t0 + 4, :]), "l0")
            chain(nc.sync.dma_start(wvo[:], wvoT), "l0")

            wk3 = wqk[:, 0:NDT * FEAT].rearrange("p (t f) -> p t f", t=NDT)
            wq3 = wqk[:, NDT * FEAT:2 * NDT * FEAT].rearrange(
                "p (t f) -> p t f", t=NDT)
            bk3 = wqk[:, 2 * NDT * FEAT:2 * NDT * FEAT + 4].bitcast(fp32)
            bq3 = wqk[:, 2 * NDT * FEAT + 4:2 * NDT * FEAT + 8].bitcast(fp32)
            wv3 = wvo[:, 0:NDT * FEAT].rearrange("p (t f) -> p t f", t=NDT)
            wo3 = wvo[:, NDT * FEAT:].rearrange("p (t j) -> p t j", t=2)

            # ---- persistent intermediates ----
            qh3 = proj.tile([128, 2, S], dt_qk, tag="qh")   # pair-packed
            kh3 = proj.tile([128, 2, S], dt_qk, tag="kh")
            vha = proj.tile([128, NKT, HL, DK + 1], dt_pv, tag="vha")
            ot3 = proj.tile([128, 2, S], dt_o, tag="outT")
            nc.gpsimd.memset(vha[:, :, :, DK], 1.0)  # ones col -> denominators

            # ---- PE p-state warmup while the first DMAs land ----
            wdum = wdum0
            wu = pp.tile([128, 512], fp32, tag="acc")
            for i in range(N_WARMUP):
                nc.tensor.matmul(wu[:], wdum[:, 0:128], wdum[:],
                                 start=(i == 0), stop=(i == N_WARMUP - 1))
            nc.vector.tensor_copy(junk[:], wu[:])

            # ---- projections: 2 psum accumulators per pass, kt-interleaved
            # so matmuls chase the chunked x DMAs ----
            def qk_pass(x3, w3, b3, dst, m, nn):
                accs = [pp.tile([128, 512], fp32, tag="acc", name=f"acc{n}")
                        for n in nn]
                for kt in range(NDT):
                    for a, n in zip(accs, nn):
                        nc.tensor.matmul(
                            a[:], w3[:, kt, m * 128:(m + 1) * 128],
                            x3[:, kt, n * 512:(n + 1) * 512],
                            start=(kt == 0), stop=(kt == NDT - 1))
                for a, n in zip(accs, nn):
                    nc.vector.tensor_scalar_add(
                        dst[:, m, n * 512:(n + 1) * 512], a[:],
                        b3[:, m:m + 1])

            def v_proj():
                for st in range(NKT):
                    ps = pp.tile([128, 512], fp32, tag="acc")
                    for kt in range(NDT):
                        nc.tensor.matmul(
                            ps[:, 0:256], xv3[:, kt, st * 128:(st + 1) * 128],
                            wv3[:, kt, :],
                            start=(kt == 0), stop=(kt == NDT - 1))
                    nc.vector.tensor_copy(vha[:, st, :, 0:DK],
                                          ps[:, 0:256])

            def attn_scores(qt, hp, e2u):
                for kt in range(NKT):
                    s2 = ps2.tile([128, 1024], fp32, tag="s2")
                    nc.tensor.matmul(
                        s2[:, 0:512],
                        kh3[0:64, hp, kt * 128:(kt + 1) * 128],
                        qh3[0:64, hp, qt * 512:(qt + 1) * 512],
                        start=True, stop=True)
                    nc.tensor.matmul(
                        s2[:, 512:1024],
                        kh3[64:128, hp, kt * 128:(kt + 1) * 128],
                        qh3[64:128, hp, qt * 512:(qt + 1) * 512],
                        start=True, stop=True)
                    nc.scalar.activation(
                        e2u[:, kt, :], s2[:],
                        mybir.ActivationFunctionType.Exp, scale=0.125)

            def attn_pv(qt, hp, e2u):
                pva = pspv.tile([DK + 1, 512], fp32, tag="pv")
                pvb = pspv.tile([DK + 1, 512], fp32, tag="pv")
                for kt in range(NKT):
                    nc.tensor.matmul(
                        pva[:], vha[:, kt, 2 * hp, :], e2u[:, kt, 0:512],
                        start=(kt == 0), stop=(kt == NKT - 1))
                    nc.tensor.matmul(
                        pvb[:], vha[:, kt, 2 * hp + 1, :],
                        e2u[:, kt, 512:1024],
                        start=(kt == 0), stop=(kt == NKT - 1))
                for pv, half in ((pva, 0), (pvb, 1)):
                    # copy the whole accumulator to SBUF first: frees the
                    # PSUM bank for the next unit's PV in ~0.7us instead of
                    # after the full normalize chain; DVE cost is the same
                    # (free-size bound) and custom DVE ops need SBUF anyway.
                    pvs = pnrm.tile([DK + 1, 512], fp32, tag="pvs")
                    nc.vector.tensor_copy(pvs[:], pv[:])
                    # custom-DVE recip needs a base-partition-0 input; BIR
                    # also requires partition starts in {0,32,64,96}, so the
                    # denominator row (partition 64) is staged via srow
                    srow = pnrm.tile([1, 512], fp32, tag="srow")
                    nc.vector.tensor_copy(srow[:], pvs[DK:DK + 1, :])
                    inv = pnrm.tile([1, 512], fp32, tag="inv")
                    nc.vector.reciprocal_approx_fast(inv[:], srow[:])
                    invb = pnrm.tile([64, 512], fp32, tag="invb")
                    nc.gpsimd.partition_broadcast(invb[:], inv[:])
                    nc.vector.tensor_tensor(
                        ot3[half * 64:(half + 1) * 64, hp,
                            qt * 512:(qt + 1) * 512],
                        pvs[0:DK, :], invb[:], mybir.AluOpType.mult)

            def oproj(qt):
                po = pout.tile([128, NDT, 512], dt_out, tag="po", bufs=1)
                for jt in range(NDT):
                    ps = pp.tile([128, 512], fp32, tag="acc")
                    for m in range(2):
                        nc.tensor.matmul(
                            ps[:], wo3[:, m, jt * 128:(jt + 1) * 128],
                            ot3[:, m, qt * 512:(qt + 1) * 512],
                            start=(m == 0), stop=(m == 1))
                    nc.vector.tensor_copy(po[:, jt, :], ps[:])
                nc.sync.dma_start(out_d[qt], po[:])

            def e2tile(name):
                return big.tile([128, NKT, 1024], dt_pv, tag="big", name=name)

            # ---- emission order == per-engine execution order ----
            for m in range(2):                       # k-proj (all pairs)
                for nn in ((0, 1), (2, 3)):
                    qk_pass(xk3, wk3, bk3, kh3, m, nn)
            qk_pass(xq3, wq3, bq3, qh3, 0, (0,))     # q-proj heads only
            qk_pass(xq3, wq3, bq3, qh3, 1, (0,))
            e00 = e2tile("e00")
            attn_scores(0, 0, e00)                   # ACT starts here
            e01 = e2tile("e01")
            attn_scores(0, 1, e01)
            for m in range(2):                       # q-proj remainder
                qk_pass(xq3, wq3, bq3, qh3, m, (1, 2))
                qk_pass(xq3, wq3, bq3, qh3, m, (3,))
            v_proj()
            attn_pv(0, 0, e00)
            e10 = e2tile("e10")
            attn_scores(1, 0, e10)
            attn_pv(0, 1, e01)
            oproj(0)
            prev = {(1, 0): e10}
            for qt in range(1, NQT):
                e_b = e2tile(f"e{qt}1")
                attn_scores(qt, 1, e_b)
                attn_pv(qt, 0, prev[(qt, 0)])
                if qt < NQT - 1:
                    e_a = e2tile(f"e{qt + 1}0")
                    attn_scores(qt + 1, 0, e_a)
                    prev[(qt + 1, 0)] = e_a
                attn_pv(qt, 1, e_b)
                if qt == NQT - 1:
                    wu2 = pp.tile([128, 512], fp32, tag="acc", name="wu2")
                    for i in range(10):
                        nc.tensor.matmul(wu2[:], wdum[:, 0:128], wdum[:],
                                         start=(i == 0), stop=(i == 9))
                    nc.vector.tensor_copy(junk[:], wu2[:])
                oproj(qt)

    nc.compile()
    return nc


def kernel(q, k, v, Wq, bq, Wk, bk, Wv, bv, Wo, bo, _trace=False):
    from concourse import bass_utils

    if "nc" not in _cache:
        _cache["nc"] = _build()
    nc = _cache["nc"]

    q = np.asarray(q, np.float32)
    k = np.asarray(k, np.float32)
    v = np.asarray(v, np.float32)
    Wq = np.asarray(Wq, np.float32)
    Wk = np.asarray(Wk, np.float32)
    Wv = np.asarray(Wv, np.float32)
    Wo = np.asarray(Wo, np.float32)
    bq = np.asarray(bq, np.float32)
    bk = np.asarray(bk, np.float32)
    bv = np.asarray(bv, np.float32)
    bo = np.asarray(bo, np.float32)

    d_qk, d_v, d_o = _np_dt(DT_QK), _np_dt(DT_V), _np_dt(DT_O)

    def swz(a, t):     # [t*128, f] -> [128, t*f], rows contiguous in DRAM
        f = a.shape[1]
        return np.ascontiguousarray(
            a.reshape(t, 128, f).transpose(1, 0, 2).reshape(128, t * f))

    xT = {}
    for b in range(B):
        xT[("q", b)] = swz(np.ascontiguousarray(q[b].T), NDT).astype(d_qk)
        xT[("k", b)] = swz(np.ascontiguousarray(k[b].T), NDT).astype(d_qk)
        xT[("v", b)] = swz(np.ascontiguousarray(v[b].T), NDT).astype(d_v)
    wT = {}
    for g in range(HG):
        sl = slice(g * FEAT, (g + 1) * FEAT)
        wk_s = swz(np.ascontiguousarray(Wk[sl, :].T), NDT).astype(d_qk)
        wq_s = swz(np.ascontiguousarray(Wq[sl, :].T), NDT).astype(d_qk)
        bk_s = np.ascontiguousarray(
            bk[sl].astype(np.float32).reshape(2, 128).T).view(np.uint16)
        bq_s = np.ascontiguousarray(
            bq[sl].astype(np.float32).reshape(2, 128).T).view(np.uint16)
        wqk = np.concatenate(
            [wk_s.view(np.uint16), wq_s.view(np.uint16), bk_s, bq_s],
            axis=1).view(d_qk)
        wv_s = swz(np.ascontiguousarray(Wv[sl, :].T), NDT).astype(d_v)
        wo_s = swz(np.ascontiguousarray(Wo[:, sl].T), 2).astype(d_o)
        wT[("qk", g)] = np.ascontiguousarray(wqk)
        wT[("vo", g)] = np.ascontiguousarray(
            np.concatenate([wv_s, wo_s], axis=1))

    in_maps = []
    for c in range(N_CORES):
        b, g = divmod(c, HG)
        in_maps.append({
            "xqT": xT[("q", b)], "xkT": xT[("k", b)], "xvT": xT[("v", b)],
            "wqkT": wT[("qk", g)], "wvoT": wT[("vo", g)],
        })

    kwargs = {}
    if _trace:
        _install_profile_shim()
        kwargs = dict(trace=True, trace_cores=list(range(N_CORES)))
    res = bass_utils.run_bass_kernel_spmd(
        nc, in_maps, core_ids=list(range(N_CORES)), **kwargs)
    _cache["last_results"] = res

    final_bias = (Wo @ bv + bo).astype(np.float32)  # attn rows sum to 1
    out = np.empty((B, S, D), np.float32)
    for b in range(B):
        acc = res.results[b * HG]["partialT"].astype(np.float32)
        for g in range(1, HG):
            acc += res.results[b * HG + g]["partialT"].astype(np.float32)
        # [qt, p, jt, s] -> [S, D]:  d = jt*128+p, q = qt*512+s
        out[b] = acc.transpose(0, 3, 2, 1).reshape(S, D) + final_bias
    return out


def _install_profile_shim():
    """Provide antenv.axon_hooks so trace=True works under axon."""
    import sys
    import types

    import antenv

    if "antenv.axon_hooks" in sys.modules:
        return
    mod = types.ModuleType("antenv.axon_hooks")
    mod._hook = None
    mod.set_axon_ntff_profile_hook = lambda h: setattr(mod, "_hook", h)
    mod.get_axon_ntff_profile_hook = lambda: mod._hook
    sys.modules["antenv.axon_hooks"] = mod
    antenv.axon_hooks = mod
    try:
        from trn_agent_boot.trn_boot import _ntff_profile_via_ctypes
        mod.set_axon_ntff_profile_hook(
            _ntff_profile_via_ctypes("/opt/axon/libaxon_pjrt.so"))
    except Exception:
        pass
